# revision 75
# baseline (speedup 1.0000x reference)
"""Trainium2 Bass kernel for nn_AttentionSeparateQKV (B=16, N=1024, D=768, H=12).

Data-parallel over batch: 8 NeuronCores x 2 batches each. Per core, per
batch, per head-pair f (128 features = heads 2f, 2f+1):
  qT/kT projections (fp32r matmuls, bias fused in PSUM->SBUF add)
  scores computed TRANSPOSED (key-major): scT[key,q] = kT_h^T @ qT_h
  exp on ScalarE (scale fused) -> probsT bf16 directly (no probs transpose,
  no accum_out)
  V = K: vf[key,d] via 8 PE transposes per stage into a shared PSUM bank
  (the XBAR bf16 transpose corrupts even partitions on this backend for
  this src pattern); AV is emitted q-major (M=128 full) with an extra
  ap-1 matmul against a ones vector per region for softmax denominators:
    av[q, 0:64] = sum_k p*v,  av[q, 64] = sum_k p
  Multi-region PSUM accumulation uses one whole-bank start=True zeroing
  matmul per bank per stage (hardware pending-zero covers the 2KB zero
  region) + start=False region accumulates with skip_group_check.
  normalize on DVE with per-partition scalars (denom is per-q now)
  attn -> feature-major outT via one XBAR DMA transpose per stage
  out-proj bf16 matmuls + replicated-bias DVE add
"""

import sys

if "/opt/trn_rl_repo" not in sys.path:
    sys.path.insert(0, "/opt/trn_rl_repo")

from collections import deque

import numpy as np

B, N, D, H = 16, 1024, 768, 12
HD = D // H                # 64
SCALE = float(HD) ** -0.5  # 0.125
N_CORES = 8
BL = B // N_CORES          # 2 batches per core
T = BL * N                 # 2048 tokens per core
FT = D // 128              # 6 feature tiles == head pairs
NKT = N // 128             # 8 key tiles per batch
NQT = N // 128             # 8 query tiles per batch

_NC_CACHE = []


def _build():
    import concourse.mybir as mybir
    import concourse.tile as tile
    from concourse import bacc

    F32 = mybir.dt.float32
    F32R = mybir.dt.float32r
    BF16 = mybir.dt.bfloat16
    EXP = mybir.ActivationFunctionType.Exp
    ADD = mybir.AluOpType.add

    # Always-on: these tiny stage-(b0,f0) DRAM dumps add readers whose
    # dependencies steer the tile scheduler into an instruction order that
    # is correct on hardware (without them a latent ordering hazard in the
    # shared-PSUM-bank accumulation produces wrong results on this backend).
    kdebug = True

    nc = bacc.Bacc("TRN2", target_bir_lowering=False, debug=False)

    x_d = nc.dram_tensor("x", [D, T], BF16, kind="ExternalInput").ap()  # host-pretransposed
    wq_d = nc.dram_tensor("wqt", [D, D], BF16, kind="ExternalInput").ap()
    wk_d = nc.dram_tensor("wkt", [D, D], BF16, kind="ExternalInput").ap()
    wp_d = nc.dram_tensor("wpt", [D, D], F32R, kind="ExternalInput").ap()
    bq_d = nc.dram_tensor("bqp", [128, FT], F32, kind="ExternalInput").ap()
    bk_d = nc.dram_tensor("bkp", [128, FT], F32, kind="ExternalInput").ap()
    bc_d = nc.dram_tensor("bc", [1, 896], F32R, kind="ExternalInput").ap()
    id_d = nc.dram_tensor("identb", [128, 128], BF16, kind="ExternalInput").ap()
    out_d = nc.dram_tensor("out", [T, D], F32, kind="ExternalOutput").ap()
    dbg = {}
    if kdebug:
        for nm, shape, dt in [
            ("dbg_qT", [128, N], F32R),
            ("dbg_kT", [128, N], F32R),
            ("dbg_k16", [128, N], BF16),
            ("dbg_vf", [128, NKT, 128], BF16),
            ("dbg_pT0", [128, N], BF16),
            ("dbg_pT1", [128, N], BF16),
            ("dbg_avsb", [128, 3, 390], F32),
            ("dbg_attn", [128, NQT, 128], BF16),
            ("dbg_outT", [128, FT, NQT, 128], BF16),
            ("dbg_brep", [128, D], F32),
            ("dbg_wp", [128, FT, D], BF16),
        ]:
            dbg[nm] = nc.dram_tensor(nm, shape, dt, kind="ExternalOutput").ap()

    x_r = x_d.rearrange("(ko kp) t -> kp ko t", kp=128)

    with tile.TileContext(nc) as tc:
        with (
            tc.tile_pool(name="const", bufs=1) as cpool,
            tc.tile_pool(name="xp", bufs=2) as xpool,
            tc.tile_pool(name="qk", bufs=2) as qkpool,
            tc.tile_pool(name="k16p", bufs=2) as k16pool,
            tc.tile_pool(name="vfp", bufs=2) as vfpool,
            tc.tile_pool(name="pTp", bufs=9) as pTpool,
            tc.tile_pool(name="avsb", bufs=2) as avsbpool,
            tc.tile_pool(name="wpp", bufs=1) as wppool,
            tc.tile_pool(name="attnp", bufs=2) as attnpool,
            tc.tile_pool(name="outTp", bufs=2) as outTpool,
            tc.tile_pool(name="finp", bufs=3) as finpool,
            tc.tile_pool(name="recipp", bufs=6) as recippool,
            tc.tile_pool(name="ps_sc", bufs=2, space="PSUM") as ps_sc,
            tc.tile_pool(name="ps_av", bufs=3, space="PSUM") as ps_av,
            tc.tile_pool(name="ps_proj", bufs=1, space="PSUM") as ps_proj,
        ):
            # ---- constants / weights (gpsimd SWDGE: keeps the scalar/ACT
            # queue free of DMA dispatches, which block exp dispatch) ----
            bq_sb = cpool.tile([128, FT], F32, tag="bq")
            bk_sb = cpool.tile([128, FT], F32, tag="bk")
            bc_sb = cpool.tile([1, 896], F32R, tag="bc")
            zc_sb = cpool.tile([1, 1024], BF16, tag="zc")
            ones_sb = cpool.tile([128, 1], BF16, tag="ones")
            id_sb = cpool.tile([128, 128], BF16, tag="ident")

            wq_sb = cpool.tile([128, FT, D], BF16, tag="wq")
            wk_sb = cpool.tile([128, FT, D], BF16, tag="wk")
            wp_sb = cpool.tile([128, FT, D], BF16, tag="wp")
            bias_rep = cpool.tile([128, D], F32, tag="brep")
            wq_r = wq_d.rearrange("(ko kp) m -> kp ko m", kp=128)
            wk_r = wk_d.rearrange("(ko kp) m -> kp ko m", kp=128)

            def emit_xT(b, sliced):
                """Load feature-major x slice for batch b (host-pretransposed).
                sliced=True loads 12 (ko, half) pieces, first-needed first."""
                xT = xpool.tile([128, FT, N], BF16, tag="xT", name="xT")
                if sliced:
                    for ko in range(FT):
                        nc.sync.dma_start(
                            xT[:, ko : ko + 1, :],
                            x_r[:, ko : ko + 1, b * N : (b + 1) * N],
                        )
                else:
                    nc.gpsimd.dma_start(xT[:], x_r[:, :, b * N : (b + 1) * N])
                return xT

            # first needed pieces first, interleaved on the fast HWDGE queue:
            # wq f0, x ko0 (first proj matmul's inputs), wk f0, rest of x
            xT0 = xpool.tile([128, FT, N], BF16, tag="xT", name="xT0")
            nc.sync.dma_start(wq_sb[:, :, 0:128], wq_r[:, :, 0:128])
            nc.sync.dma_start(xT0[:, 0:1, :], x_r[:, 0:1, 0:N])
            nc.sync.dma_start(wk_sb[:, :, 0:128], wk_r[:, :, 0:128])
            for ko in range(1, FT):
                nc.sync.dma_start(
                    xT0[:, ko : ko + 1, :], x_r[:, ko : ko + 1, 0:N]
                )
            nc.gpsimd.dma_start(bq_sb[:], bq_d[:])
            nc.gpsimd.dma_start(bk_sb[:], bk_d[:])
            nc.gpsimd.dma_start(bc_sb[:], bc_d[:])
            nc.gpsimd.dma_start(wq_sb[:, :, 128:256], wq_r[:, :, 128:256])
            nc.gpsimd.dma_start(wk_sb[:, :, 128:256], wk_r[:, :, 128:256])
            nc.gpsimd.dma_start(id_sb[:], id_d[:])
            # exact constants built from loaded data (no memset dependence):
            # zc = bc*0 (zeros), ones = bq*0 + 1
            nc.vector.tensor_scalar_mul(zc_sb[:, 0:896], bc_sb[0:1, 0:896], 0.0)
            nc.vector.tensor_scalar_mul(zc_sb[:, 896:1024], bc_sb[0:1, 0:128], 0.0)
            nc.vector.tensor_scalar(
                ones_sb[:], bq_sb[:, 0:1], 0.0, 1.0,
                mybir.AluOpType.mult, mybir.AluOpType.add,
            )

            def emit_weight_rest():
                """Deferred f2..f5 weight slices — emitted mid-stage-f0 so the
                f0 v-transposes win the DMA queue race."""
                for wf in range(2, FT):
                    sl = slice(128 * wf, 128 * (wf + 1))
                    nc.gpsimd.dma_start(wq_sb[:, :, sl], wq_r[:, :, sl])
                    nc.gpsimd.dma_start(wk_sb[:, :, sl], wk_r[:, :, sl])

            wp_r = wp_d.rearrange("(ko kp) m -> kp ko m", kp=128)

            def emit_wp_load():
                """Deferred wp load + bf16 convert (needed only by the first
                out-projection, ~100us in). Converts run on the idle Pool
                engine so the DVE queue stays clear; the staging tile holds
                half of wp at a time."""
                for h in range(2):
                    wp32 = wppool.tile([128, 3, D], F32R, tag="wp32", name="wp32")
                    nc.gpsimd.dma_start(wp32[:], wp_r[:, 3 * h : 3 * (h + 1), :])
                    for ks in range(3):
                        nc.gpsimd.tensor_copy(
                            wp_sb[:, 3 * h + ks, :], wp32[:, ks, :]
                        )

            # ---- stage-state helpers -------------------------------------
            pend_av = deque()   # deferred AV-group closures
            pending = deque()   # deferred out-projection piece args (outT, b, tt, ns)
            fin_tiles = {}

            def emit_proj_chunk(xT, f, which, c, qt_, kt_):
                """One projection chunk: 6 matmuls + bias add for tokens
                [512c, 512c+512) of q (which=0) or k (which=1)."""
                w_sb = wq_sb if which == 0 else wk_sb
                b_sb = bq_sb if which == 0 else bk_sb
                dst = qt_ if which == 0 else kt_
                pp = ps_proj.tile([128, 512], F32, tag="pp", name="pp")
                for ks in range(FT):
                    nc.tensor.matmul(
                        pp[:],
                        w_sb[:, ks, 128 * f : 128 * (f + 1)],
                        xT[:, ks, 512 * c : 512 * (c + 1)],
                        start=(ks == 0),
                        stop=(ks == FT - 1),
                    )
                nc.vector.tensor_scalar_add(
                    dst[:, 512 * c : 512 * (c + 1)], pp[:], b_sb[:, f : f + 1]
                )

            def emit_v_half(kt_, k16, vf, c):
                """bf16 copy of kT half c (transpose happens separately on PE;
                the XBAR bf16 transpose corrupts even partitions on this HW)."""
                nc.vector.tensor_copy(
                    k16[:, 512 * c : 512 * (c + 1)], kt_[:, 512 * c : 512 * (c + 1)]
                )

            def emit_vf_build(k16, vf):
                """vf[key, kt, d] = k16[d, kt*128+key] via 8 PE transposes into
                one shared PSUM bank (zero-anchored), then one copy to SBUF."""
                vfT = ps_proj.tile([128, 512], F32, tag="pp", name="vfT")
                nc.tensor.matmul(
                    vfT[:], zc_sb[0:1, 0:128], zc_sb[0:1, 0:512],
                    start=True, stop=False, skip_group_check=True,
                )
                vfT16 = vfT.bitcast(BF16)
                for kt in range(NKT):
                    nc.tensor.matmul(
                        vfT16[:, 128 * kt : 128 * (kt + 1)],
                        k16[:, 128 * kt : 128 * (kt + 1)],
                        id_sb[:],
                        is_transpose=True,
                        start=False, stop=False, skip_group_check=True,
                    )
                nc.vector.tensor_copy(vf[:], vfT16[:])

            # (vf tiles are flat [128, 1024]: cols = kt*128 + d)

            def emit_outproj_piece(outT, b, tt, ns, kind=0):
                """Half (384 cols) of the final projection for token tile tt.
                kind=0 uses the shared proj bank; kind=1 borrows a scores
                tile (only safe in the tail when scoring is finished)."""
                if ns == 0:
                    fin_tiles[(b, tt)] = finpool.tile([128, D], F32, tag="fin", name="fin")
                fin = fin_tiles[(b, tt)]
                if kind == 0:
                    pf = ps_proj.tile([128, 512], F32, tag="pp", name="pf")
                else:
                    pf = ps_sc.tile([128, 1024], F32, tag="sc", name="pf")
                for ks in range(FT):
                    nc.tensor.matmul(
                        pf[:, 0:384],
                        outT[:, ks, tt, :],
                        wp_sb[:, ks, 384 * ns : 384 * (ns + 1)],
                        start=(ks == 0),
                        stop=(ks == FT - 1),
                    )
                nc.vector.tensor_tensor(
                    fin[:, 384 * ns : 384 * (ns + 1)],
                    pf[:, 0:384],
                    bias_rep[:, 384 * ns : 384 * (ns + 1)],
                    ADD,
                )
                if ns == 1:
                    nc.sync.dma_start(
                        out_d[b * N + 128 * tt : b * N + 128 * (tt + 1), :], fin[:]
                    )

            QT_GROUPS = ((0, 1, 2), (3, 4, 5), (6, 7))

            def emit_av_group(kt, pT0, pT1, vf, av_tiles, fin_ctx):
                """AV matmuls for key tile kt (all query tiles, both heads).
                The bank was zeroed by a start=True whole-bank matmul at stage
                start (hardware pending-zero covers the full 2KB region), so
                every region accumulates with start=False; the group check is
                skipped since regions share the bank's zero region.
                On the last key tile, evacuate PSUM to SBUF (releases the
                av banks fast) and emit normalization + transpose."""
                pTs = (pT0, pT1)
                for qt in range(NQT):
                    g, r = qt // 3, qt % 3
                    for e in range(2):
                        base = 130 * r + 65 * e
                        nc.tensor.matmul(
                            av_tiles[g][:, base : base + 64],
                            pTs[e][:, 128 * qt : 128 * (qt + 1)],
                            vf[:, 128 * kt + 64 * e : 128 * kt + 64 * e + 64],
                            start=False,
                            stop=False,
                            skip_group_check=True,
                        )
                        nc.tensor.matmul(
                            av_tiles[g][:, base + 64 : base + 65],
                            pTs[e][:, 128 * qt : 128 * (qt + 1)],
                            ones_sb[:, 0:1],
                            start=False,
                            stop=False,
                            skip_group_check=True,
                        )
                if kt == NKT - 1:
                    av_sb = avsbpool.tile([128, 3, 390], F32, tag="avsb", name="av_sb")
                    nc.vector.tensor_copy(av_sb[:, 0, :], av_tiles[0][:])
                    nc.vector.tensor_copy(av_sb[:, 1, :], av_tiles[1][:])
                    nc.vector.tensor_copy(av_sb[:, 2, 0:260], av_tiles[2][:, 0:260])
                    if kdebug and fin_ctx[0] == 0 and fin_ctx[1] == 0:
                        nc.sync.dma_start(dbg["dbg_avsb"][:, 0:2, :], av_sb[:, 0:2, :])
                        nc.sync.dma_start(
                            dbg["dbg_avsb"][:, 2, 0:260], av_sb[:, 2, 0:260]
                        )
                    emit_normalize(av_sb, fin_ctx)

            def emit_normalize(av_sb, fin_ctx):
                """Per-stage softmax normalization + attn transpose to outT.
                On the very last stage, transpose per query tile so the tail
                out-projection can start as soon as its tokens are ready."""
                b, f, outT = fin_ctx
                last = False  # per-qt [128,128] transposes misbehave on HW
                attn = attnpool.tile([128, NQT, 128], BF16, tag="attn", name="attn")
                recips = []
                for g, qts in enumerate(QT_GROUPS):
                    cnt = 2 * len(qts)
                    rc = recippool.tile([128, 6], F32, tag="rc", name="rc")
                    nc.vector.reciprocal(
                        rc[:, 0:cnt], av_sb[:, g, 64 : 64 + 65 * (cnt - 1) + 1 : 65]
                    )
                    recips.append(rc)
                for qt in range(NQT):
                    g, r = qt // 3, qt % 3
                    for e in range(2):
                        nc.vector.tensor_scalar_mul(
                            attn[:, qt, 64 * e : 64 * (e + 1)],
                            av_sb[:, g, 130 * r + 65 * e : 130 * r + 65 * e + 64],
                            recips[g][:, 2 * r + e : 2 * r + e + 1],
                        )
                    if last:
                        nc.sync.dma_start(
                            outT[:, f, qt : qt + 1, :],
                            attn[:, qt, :],
                            transpose=True,
                        )
                if not last:
                    nc.sync.dma_start(outT[:, f, :, :], attn[:], transpose=True)
                if kdebug and b == 0 and f == 0:
                    nc.sync.dma_start(dbg["dbg_attn"][:], attn[:])
                if kdebug and b == 0 and f == FT - 1:
                    nc.sync.dma_start(dbg["dbg_outT"][:], outT[:])
                    nc.sync.dma_start(dbg["dbg_brep"][:], bias_rep[:])
                    nc.sync.dma_start(dbg["dbg_wp"][:], wp_sb[:])
                if f == FT - 1:
                    pending.extend((outT, b, tt, ns) for tt in range(NQT) for ns in range(2))

            # ---- prologue: proj (b0, f0), bias replicate -----------------
            def emit_proj_full(xT, f):
                qt_ = qkpool.tile([128, N], F32R, tag="q", name="qTf")
                kt_ = qkpool.tile([128, N], F32R, tag="k", name="kTf")
                k16 = k16pool.tile([128, N], BF16, tag="k16", name="k16")
                vf = vfpool.tile([128, NKT * 128], BF16, tag="vf", name="vf")
                return qt_, kt_, k16, vf

            cur = emit_proj_full(xT0, 0)
            for c in range(2):
                emit_proj_chunk(xT0, 0, 0, c, cur[0], cur[1])
                emit_proj_chunk(xT0, 0, 1, c, cur[0], cur[1])
                emit_v_half(cur[1], cur[2], cur[3], c)
            emit_vf_build(cur[2], cur[3])

            for ns in range(2):
                pb = ps_proj.tile([128, 512], F32, tag="pp", name="pb")
                nc.tensor.matmul(
                    pb[:, 0:384],
                    bc_sb[0:1, 768:896],
                    bc_sb[0:1, 384 * ns : 384 * (ns + 1)],
                    start=True,
                    stop=True,
                )
                nc.vector.tensor_copy(bias_rep[:, 384 * ns : 384 * (ns + 1)], pb[:, 0:384])

            # ---- main loop ----------------------------------------------
            xT = xT0
            xT_next = None
            for b in range(BL):
                if b > 0:
                    xT = xT_next
                outT = outTpool.tile([128, FT, NQT, 128], BF16, tag="outT", name="outT")
                for f in range(FT):
                    qt_, kt_, k16, vf = cur
                    av_tiles = [
                        ps_av.tile([128, 390], F32, tag="av", name="av") for _ in range(3)
                    ]
                    # zero each av bank with one whole-bank start=True matmul
                    # (0 * anything); marks the 2KB zero region pending-zero and
                    # anchors a real write-dep for the start=False accumulates.
                    for g in range(3):
                        nc.tensor.matmul(
                            av_tiles[g][:],
                            zc_sb[0:1, 0:128],
                            zc_sb[0:1, 0:390],
                            start=True,
                            stop=False,
                            skip_group_check=True,
                        )
                    fin_ctx = (b, f, outT)

                    # filler schedule for this stage (next-stage projections
                    # and pending out-projection pieces)
                    fillers = deque()
                    nf = f + 1
                    if nf < FT or b + 1 < BL:
                        nxt_b = b if nf < FT else b + 1
                        nxt_f = nf % FT
                        nxt_x = xT if nf < FT else xT_next
                        nxt = emit_proj_full(nxt_x, nxt_f)
                        for c in range(2):
                            fillers.append(
                                (lambda c=c, nx=nxt_x, nf_=nxt_f, t=nxt: emit_proj_chunk(
                                    nx, nf_, 0, c, t[0], t[1]))
                            )
                            fillers.append(
                                (lambda c=c, nx=nxt_x, nf_=nxt_f, t=nxt: (
                                    emit_proj_chunk(nx, nf_, 1, c, t[0], t[1]),
                                    emit_v_half(t[1], t[2], t[3], c)))
                            )
                        fillers.append(lambda t=nxt: emit_vf_build(t[2], t[3]))
                    else:
                        nxt = None

                    if b == 0 and f == 1:
                        emit_wp_load()
                    if b == 0 and f == 2:
                        xT_next = emit_xT(1, sliced=False)

                    exp_split = b == 0 and f == 0  # ramp ACT before x c1 lands
                    if kdebug and b == 0 and f == 0:
                        nc.sync.dma_start(dbg["dbg_qT"][:], qt_[:])
                        nc.sync.dma_start(dbg["dbg_kT"][:], kt_[:])
                        nc.sync.dma_start(dbg["dbg_k16"][:], k16[:])
                        nc.sync.dma_start(dbg["dbg_vf"][:], vf[:])
                    for kt in range(NKT):
                        if b == 0 and f == 0 and kt == 4:
                            emit_weight_rest()
                        pT0 = pTpool.tile([128, N], BF16, tag="pT", name="pT0")
                        pT1 = pTpool.tile([128, N], BF16, tag="pT", name="pT1")
                        pTs = (pT0, pT1)
                        for e in range(2):
                            sct = ps_sc.tile([128, 1024], F32, tag="sc", name="sct")
                            for c in range(2):
                                nc.tensor.matmul(
                                    sct[:, 512 * c : 512 * (c + 1)],
                                    kt_[64 * e : 64 * (e + 1), 128 * kt : 128 * (kt + 1)],
                                    qt_[64 * e : 64 * (e + 1), 512 * c : 512 * (c + 1)],
                                    start=True,
                                    stop=True,
                                )
                                if exp_split:
                                    nc.scalar.activation(
                                        pTs[e][:, 512 * c : 512 * (c + 1)],
                                        sct[:, 512 * c : 512 * (c + 1)],
                                        EXP,
                                        scale=SCALE,
                                    )
                            if not exp_split:
                                nc.scalar.activation(pTs[e][:], sct[:], EXP, scale=SCALE)
                        if kdebug and b == 0 and f == 0 and kt == 0:
                            nc.sync.dma_start(dbg["dbg_pT0"][:], pT0[:])
                            nc.sync.dma_start(dbg["dbg_pT1"][:], pT1[:])
                        # mid-step filler keeps PE fed while ACT chews exps.
                        # Keep 5 pieces in reserve: they cover the tail's
                        # PE idle while the last stage normalizes.
                        if fillers:
                            fillers.popleft()()
                        elif len(pending) > (4 if (b == BL - 1 and f == FT - 1) else 12):
                            emit_outproj_piece(*pending.popleft())
                        if len(pend_av) > 2:
                            pend_av.popleft()()
                        pend_av.append(
                            lambda kt=kt, pT0=pT0, pT1=pT1, vf=vf, av=av_tiles, fc=fin_ctx: emit_av_group(
                                kt, pT0, pT1, vf, av, fc
                            )
                        )
                    cur = nxt

            # ---- tail ----------------------------------------------------
            while pend_av:
                pend_av.popleft()()
            flip = 0
            while pending:
                emit_outproj_piece(*pending.popleft(), kind=0 if flip == 0 else 1)
                flip = (flip + 1) % 3

    nc.compile()
    return nc


def _get_nc():
    if not _NC_CACHE:
        _NC_CACHE.append(_build())
    return _NC_CACHE[0]


def _to_np(a):
    try:
        return np.asarray(a)
    except Exception:
        import jax

        return np.asarray(jax.device_get(a))


def _prep_inputs(x, Wq, bq, Wk, bk, Wp, bp):
    import ml_dtypes

    x, Wq, bq, Wk, bk, Wp, bp = (_to_np(a) for a in (x, Wq, bq, Wk, bk, Wp, bp))
    x = np.ascontiguousarray(np.asarray(x, dtype=np.float32))
    wqt = np.ascontiguousarray(np.asarray(Wq, np.float32).T.astype(ml_dtypes.bfloat16))
    wkt = np.ascontiguousarray(np.asarray(Wk, np.float32).T.astype(ml_dtypes.bfloat16))
    wpt = np.ascontiguousarray(np.asarray(Wp, np.float32).T)
    bqp = np.ascontiguousarray(np.asarray(bq, np.float32).reshape(FT, 128).T)
    bkp = np.ascontiguousarray(np.asarray(bk, np.float32).reshape(FT, 128).T)
    bc = np.zeros((1, 896), np.float32)
    bc[0, :D] = np.asarray(bp, np.float32)
    bc[0, D : D + 128] = 1.0
    import ml_dtypes

    identb = np.eye(128, dtype=ml_dtypes.bfloat16)
    base = {
        "wqt": wqt, "wkt": wkt, "wpt": wpt,
        "bqp": bqp, "bkp": bkp, "bc": bc, "identb": identb,
    }
    in_maps = []
    for c in range(N_CORES):
        xc = np.ascontiguousarray(
            x[BL * c : BL * (c + 1)].reshape(T, D).T.astype(ml_dtypes.bfloat16)
        )
        in_maps.append(dict(base, x=xc))
    return in_maps


def _run(in_maps, **kw):
    from concourse.bass_utils import run_bass_kernel_spmd

    nc = _get_nc()
    return run_bass_kernel_spmd(nc, in_maps, core_ids=list(range(N_CORES)), **kw)


def kernel(x, Wq, bq, Wk, bk, Wp, bp):
    in_maps = _prep_inputs(x, Wq, bq, Wk, bk, Wp, bp)
    res = _run(in_maps)
    out = np.concatenate(
        [r["out"].reshape(BL, N, D) for r in res.results], axis=0
    )
    return out.astype(np.float32)


# revision 77
# speedup vs baseline: 1.0065x; 1.0065x over previous
"""Trainium2 Bass kernel for nn_AttentionSeparateQKV (B=16, N=1024, D=768, H=12).

Data-parallel over batch: 8 NeuronCores x 2 batches each. Per core, per
batch, per head-pair f (128 features = heads 2f, 2f+1):
  qT/kT projections (fp32r matmuls, bias fused in PSUM->SBUF add)
  scores computed TRANSPOSED (key-major): scT[key,q] = kT_h^T @ qT_h
  exp on ScalarE (scale fused) -> probsT bf16 directly (no probs transpose,
  no accum_out)
  V = K: vf[key,d] via 8 PE transposes per stage into a shared PSUM bank
  (the XBAR bf16 transpose corrupts even partitions on this backend for
  this src pattern); AV is emitted q-major (M=128 full) with an extra
  ap-1 matmul against a ones vector per region for softmax denominators:
    av[q, 0:64] = sum_k p*v,  av[q, 64] = sum_k p
  Multi-region PSUM accumulation uses one whole-bank start=True zeroing
  matmul per bank per stage (hardware pending-zero covers the 2KB zero
  region) + start=False region accumulates with skip_group_check.
  normalize on DVE with per-partition scalars (denom is per-q now)
  attn -> feature-major outT via one XBAR DMA transpose per stage
  out-proj bf16 matmuls + replicated-bias DVE add
"""

import sys

if "/opt/trn_rl_repo" not in sys.path:
    sys.path.insert(0, "/opt/trn_rl_repo")

from collections import deque

import numpy as np

B, N, D, H = 16, 1024, 768, 12
HD = D // H                # 64
SCALE = float(HD) ** -0.5  # 0.125
N_CORES = 8
BL = B // N_CORES          # 2 batches per core
T = BL * N                 # 2048 tokens per core
FT = D // 128              # 6 feature tiles == head pairs
NKT = N // 128             # 8 key tiles per batch
NQT = N // 128             # 8 query tiles per batch

_NC_CACHE = []


def _build():
    import concourse.mybir as mybir
    import concourse.tile as tile
    from concourse import bacc

    F32 = mybir.dt.float32
    F32R = mybir.dt.float32r
    BF16 = mybir.dt.bfloat16
    EXP = mybir.ActivationFunctionType.Exp
    ADD = mybir.AluOpType.add

    # Always-on: these tiny stage-(b0,f0) DRAM dumps add readers whose
    # dependencies steer the tile scheduler into an instruction order that
    # is correct on hardware (without them a latent ordering hazard in the
    # shared-PSUM-bank accumulation produces wrong results on this backend).
    kdebug = True

    nc = bacc.Bacc("TRN2", target_bir_lowering=False, debug=False)

    x_d = nc.dram_tensor("x", [D, T], BF16, kind="ExternalInput").ap()  # host-pretransposed
    wq_d = nc.dram_tensor("wqt", [D, D], BF16, kind="ExternalInput").ap()
    wk_d = nc.dram_tensor("wkt", [D, D], BF16, kind="ExternalInput").ap()
    wp_d = nc.dram_tensor("wpt", [D, D], F32R, kind="ExternalInput").ap()
    bq_d = nc.dram_tensor("bqp", [128, FT], F32, kind="ExternalInput").ap()
    bk_d = nc.dram_tensor("bkp", [128, FT], F32, kind="ExternalInput").ap()
    bc_d = nc.dram_tensor("bc", [1, 896], F32R, kind="ExternalInput").ap()
    id_d = nc.dram_tensor("identb", [128, 128], BF16, kind="ExternalInput").ap()
    out_d = nc.dram_tensor("out", [T, D], F32, kind="ExternalOutput").ap()
    dbg = {}
    if kdebug:
        for nm, shape, dt in [
            ("dbg_qT", [128, N], F32R),
            ("dbg_kT", [128, N], F32R),
            ("dbg_k16", [128, N], BF16),
            ("dbg_vf", [128, NKT, 128], BF16),
            ("dbg_pT0", [128, N], BF16),
            ("dbg_pT1", [128, N], BF16),
            ("dbg_avsb", [128, 3, 390], F32),
            ("dbg_attn", [128, NQT, 128], BF16),
            ("dbg_outT", [128, FT, NQT, 128], BF16),
            ("dbg_brep", [128, D], F32),
            ("dbg_wp", [128, FT, D], BF16),
        ]:
            dbg[nm] = nc.dram_tensor(nm, shape, dt, kind="ExternalOutput").ap()

    x_r = x_d.rearrange("(ko kp) t -> kp ko t", kp=128)

    with tile.TileContext(nc) as tc:
        with (
            tc.tile_pool(name="const", bufs=1) as cpool,
            tc.tile_pool(name="xp", bufs=2) as xpool,
            tc.tile_pool(name="qk", bufs=2) as qkpool,
            tc.tile_pool(name="k16p", bufs=2) as k16pool,
            tc.tile_pool(name="vfp", bufs=2) as vfpool,
            tc.tile_pool(name="pTp", bufs=9) as pTpool,
            tc.tile_pool(name="avsb", bufs=2) as avsbpool,
            tc.tile_pool(name="wpp", bufs=1) as wppool,
            tc.tile_pool(name="attnp", bufs=2) as attnpool,
            tc.tile_pool(name="outTp", bufs=2) as outTpool,
            tc.tile_pool(name="finp", bufs=3) as finpool,
            tc.tile_pool(name="recipp", bufs=6) as recippool,
            tc.tile_pool(name="ps_sc", bufs=2, space="PSUM") as ps_sc,
            tc.tile_pool(name="ps_av", bufs=3, space="PSUM") as ps_av,
            tc.tile_pool(name="ps_proj", bufs=1, space="PSUM") as ps_proj,
        ):
            # ---- constants / weights (gpsimd SWDGE: keeps the scalar/ACT
            # queue free of DMA dispatches, which block exp dispatch) ----
            bq_sb = cpool.tile([128, FT], F32, tag="bq")
            bk_sb = cpool.tile([128, FT], F32, tag="bk")
            bc_sb = cpool.tile([1, 896], F32R, tag="bc")
            zc_sb = cpool.tile([1, 1024], BF16, tag="zc")
            ones_sb = cpool.tile([128, 1], BF16, tag="ones")
            id_sb = cpool.tile([128, 128], BF16, tag="ident")

            wq_sb = cpool.tile([128, FT, D], BF16, tag="wq")
            wk_sb = cpool.tile([128, FT, D], BF16, tag="wk")
            wp_sb = cpool.tile([128, FT, D], BF16, tag="wp")
            bias_rep = cpool.tile([128, D], F32, tag="brep")
            wq_r = wq_d.rearrange("(ko kp) m -> kp ko m", kp=128)
            wk_r = wk_d.rearrange("(ko kp) m -> kp ko m", kp=128)

            def emit_xT(b, sliced):
                """Load feature-major x slice for batch b (host-pretransposed).
                sliced=True loads 12 (ko, half) pieces, first-needed first."""
                xT = xpool.tile([128, FT, N], BF16, tag="xT", name="xT")
                if sliced:
                    for ko in range(FT):
                        nc.sync.dma_start(
                            xT[:, ko : ko + 1, :],
                            x_r[:, ko : ko + 1, b * N : (b + 1) * N],
                        )
                else:
                    nc.gpsimd.dma_start(xT[:], x_r[:, :, b * N : (b + 1) * N])
                return xT

            # first needed pieces first, interleaved on the fast HWDGE queue:
            # wq f0, x ko0 (first proj matmul's inputs), wk f0, rest of x
            xT0 = xpool.tile([128, FT, N], BF16, tag="xT", name="xT0")
            nc.sync.dma_start(wq_sb[:, :, 0:128], wq_r[:, :, 0:128])
            nc.sync.dma_start(xT0[:, 0:1, :], x_r[:, 0:1, 0:N])
            nc.sync.dma_start(wk_sb[:, :, 0:128], wk_r[:, :, 0:128])
            for ko in range(1, FT):
                nc.sync.dma_start(
                    xT0[:, ko : ko + 1, :], x_r[:, ko : ko + 1, 0:N]
                )
            nc.gpsimd.dma_start(bq_sb[:], bq_d[:])
            nc.gpsimd.dma_start(bk_sb[:], bk_d[:])
            nc.gpsimd.dma_start(bc_sb[:], bc_d[:])
            nc.gpsimd.dma_start(wq_sb[:, :, 128:256], wq_r[:, :, 128:256])
            nc.gpsimd.dma_start(wk_sb[:, :, 128:256], wk_r[:, :, 128:256])
            nc.gpsimd.dma_start(id_sb[:], id_d[:])
            # exact constants built from loaded data (no memset dependence):
            # zc = bc*0 (zeros), ones = bq*0 + 1
            nc.vector.tensor_scalar_mul(zc_sb[:, 0:896], bc_sb[0:1, 0:896], 0.0)
            nc.vector.tensor_scalar_mul(zc_sb[:, 896:1024], bc_sb[0:1, 0:128], 0.0)
            nc.vector.tensor_scalar(
                ones_sb[:], bq_sb[:, 0:1], 0.0, 1.0,
                mybir.AluOpType.mult, mybir.AluOpType.add,
            )

            def emit_weight_rest():
                """Deferred f2..f5 weight slices — emitted mid-stage-f0 so the
                f0 v-transposes win the DMA queue race."""
                for wf in range(2, FT):
                    sl = slice(128 * wf, 128 * (wf + 1))
                    nc.gpsimd.dma_start(wq_sb[:, :, sl], wq_r[:, :, sl])
                    nc.gpsimd.dma_start(wk_sb[:, :, sl], wk_r[:, :, sl])

            wp_r = wp_d.rearrange("(ko kp) m -> kp ko m", kp=128)

            def emit_wp_load():
                """Deferred wp load + bf16 convert (needed only by the first
                out-projection, ~100us in). Converts run on the idle Pool
                engine so the DVE queue stays clear; the staging tile holds
                half of wp at a time."""
                for h in range(2):
                    wp32 = wppool.tile([128, 3, D], F32R, tag="wp32", name="wp32")
                    nc.gpsimd.dma_start(wp32[:], wp_r[:, 3 * h : 3 * (h + 1), :])
                    for ks in range(3):
                        nc.gpsimd.tensor_copy(
                            wp_sb[:, 3 * h + ks, :], wp32[:, ks, :]
                        )

            # ---- stage-state helpers -------------------------------------
            pend_av = deque()   # deferred AV-group closures
            pending = deque()   # deferred out-projection piece args (outT, b, tt, ns)
            fin_tiles = {}

            def emit_proj_chunk(xT, f, which, c, qt_, kt_):
                """One projection chunk: 6 matmuls + bias add for tokens
                [512c, 512c+512) of q (which=0) or k (which=1)."""
                w_sb = wq_sb if which == 0 else wk_sb
                b_sb = bq_sb if which == 0 else bk_sb
                dst = qt_ if which == 0 else kt_
                pp = ps_proj.tile([128, 512], F32, tag="pp", name="pp")
                for ks in range(FT):
                    nc.tensor.matmul(
                        pp[:],
                        w_sb[:, ks, 128 * f : 128 * (f + 1)],
                        xT[:, ks, 512 * c : 512 * (c + 1)],
                        start=(ks == 0),
                        stop=(ks == FT - 1),
                    )
                nc.vector.tensor_scalar_add(
                    dst[:, 512 * c : 512 * (c + 1)], pp[:], b_sb[:, f : f + 1]
                )

            def emit_v_half(kt_, k16, vf, c):
                """bf16 copy of kT half c (transpose happens separately on PE;
                the XBAR bf16 transpose corrupts even partitions on this HW)."""
                nc.vector.tensor_copy(
                    k16[:, 512 * c : 512 * (c + 1)], kt_[:, 512 * c : 512 * (c + 1)]
                )

            def emit_vf_build(k16, vf):
                """vf[key, kt, d] = k16[d, kt*128+key] via 8 PE transposes into
                one shared PSUM bank (zero-anchored), then one copy to SBUF."""
                vfT = ps_proj.tile([128, 512], F32, tag="pp", name="vfT")
                nc.tensor.matmul(
                    vfT[:], zc_sb[0:1, 0:128], zc_sb[0:1, 0:512],
                    start=True, stop=False, skip_group_check=True,
                )
                vfT16 = vfT.bitcast(BF16)
                for kt in range(NKT):
                    nc.tensor.matmul(
                        vfT16[:, 128 * kt : 128 * (kt + 1)],
                        k16[:, 128 * kt : 128 * (kt + 1)],
                        id_sb[:],
                        is_transpose=True,
                        start=False, stop=False, skip_group_check=True,
                    )
                nc.vector.tensor_copy(vf[:], vfT16[:])

            # (vf tiles are flat [128, 1024]: cols = kt*128 + d)

            def emit_outproj_piece(outT, b, tt, ns, kind=0):
                """Half (384 cols) of the final projection for token tile tt.
                kind=0 uses the shared proj bank; kind=1 borrows a scores
                tile (only safe in the tail when scoring is finished)."""
                if ns == 0:
                    fin_tiles[(b, tt)] = finpool.tile([128, D], F32, tag="fin", name="fin")
                fin = fin_tiles[(b, tt)]
                if kind == 0:
                    pf = ps_proj.tile([128, 512], F32, tag="pp", name="pf")
                elif kind == 1:
                    pf = ps_sc.tile([128, 1024], F32, tag="sc", name="pf")
                else:
                    # tail only: av banks are idle once the last avsb copies
                    # finish; their skip_group_check accumulates never set
                    # started_view, so a fresh start=True group is legal.
                    pf = ps_av.tile([128, 390], F32, tag="av", name="pf")
                for ks in range(FT):
                    nc.tensor.matmul(
                        pf[:, 0:384],
                        outT[:, ks, tt, :],
                        wp_sb[:, ks, 384 * ns : 384 * (ns + 1)],
                        start=(ks == 0),
                        stop=(ks == FT - 1),
                    )
                nc.vector.tensor_tensor(
                    fin[:, 384 * ns : 384 * (ns + 1)],
                    pf[:, 0:384],
                    bias_rep[:, 384 * ns : 384 * (ns + 1)],
                    ADD,
                )
                if ns == 1:
                    nc.sync.dma_start(
                        out_d[b * N + 128 * tt : b * N + 128 * (tt + 1), :], fin[:]
                    )

            QT_GROUPS = ((0, 1, 2), (3, 4, 5), (6, 7))

            def emit_av_group(kt, pT0, pT1, vf, av_tiles, fin_ctx):
                """AV matmuls for key tile kt (all query tiles, both heads).
                The bank was zeroed by a start=True whole-bank matmul at stage
                start (hardware pending-zero covers the full 2KB region), so
                every region accumulates with start=False; the group check is
                skipped since regions share the bank's zero region.
                On the last key tile, evacuate PSUM to SBUF (releases the
                av banks fast) and emit normalization + transpose."""
                pTs = (pT0, pT1)
                for qt in range(NQT):
                    g, r = qt // 3, qt % 3
                    for e in range(2):
                        base = 130 * r + 65 * e
                        nc.tensor.matmul(
                            av_tiles[g][:, base : base + 64],
                            pTs[e][:, 128 * qt : 128 * (qt + 1)],
                            vf[:, 128 * kt + 64 * e : 128 * kt + 64 * e + 64],
                            start=False,
                            stop=False,
                            skip_group_check=True,
                        )
                        nc.tensor.matmul(
                            av_tiles[g][:, base + 64 : base + 65],
                            pTs[e][:, 128 * qt : 128 * (qt + 1)],
                            ones_sb[:, 0:1],
                            start=False,
                            stop=False,
                            skip_group_check=True,
                        )
                if kt == NKT - 1:
                    av_sb = avsbpool.tile([128, 3, 390], F32, tag="avsb", name="av_sb")
                    nc.vector.tensor_copy(av_sb[:, 0, :], av_tiles[0][:])
                    nc.vector.tensor_copy(av_sb[:, 1, :], av_tiles[1][:])
                    nc.vector.tensor_copy(av_sb[:, 2, 0:260], av_tiles[2][:, 0:260])
                    if kdebug and fin_ctx[0] == 0 and fin_ctx[1] == 0:
                        nc.sync.dma_start(dbg["dbg_avsb"][:, 0:2, :], av_sb[:, 0:2, :])
                        nc.sync.dma_start(
                            dbg["dbg_avsb"][:, 2, 0:260], av_sb[:, 2, 0:260]
                        )
                    emit_normalize(av_sb, fin_ctx)

            def emit_normalize(av_sb, fin_ctx):
                """Per-stage softmax normalization + attn transpose to outT.
                On the very last stage, transpose per query tile so the tail
                out-projection can start as soon as its tokens are ready."""
                b, f, outT = fin_ctx
                last = False  # per-qt [128,128] transposes misbehave on HW
                attn = attnpool.tile([128, NQT, 128], BF16, tag="attn", name="attn")
                recips = []
                for g, qts in enumerate(QT_GROUPS):
                    cnt = 2 * len(qts)
                    rc = recippool.tile([128, 6], F32, tag="rc", name="rc")
                    nc.vector.reciprocal(
                        rc[:, 0:cnt], av_sb[:, g, 64 : 64 + 65 * (cnt - 1) + 1 : 65]
                    )
                    recips.append(rc)
                for qt in range(NQT):
                    g, r = qt // 3, qt % 3
                    for e in range(2):
                        nc.vector.tensor_scalar_mul(
                            attn[:, qt, 64 * e : 64 * (e + 1)],
                            av_sb[:, g, 130 * r + 65 * e : 130 * r + 65 * e + 64],
                            recips[g][:, 2 * r + e : 2 * r + e + 1],
                        )
                    if last:
                        nc.sync.dma_start(
                            outT[:, f, qt : qt + 1, :],
                            attn[:, qt, :],
                            transpose=True,
                        )
                if not last:
                    nc.sync.dma_start(outT[:, f, :, :], attn[:], transpose=True)
                if kdebug and b == 0 and f == 0:
                    nc.sync.dma_start(dbg["dbg_attn"][:], attn[:])
                if kdebug and b == 0 and f == FT - 1:
                    nc.sync.dma_start(dbg["dbg_outT"][:], outT[:])
                    nc.sync.dma_start(dbg["dbg_brep"][:], bias_rep[:])
                    nc.sync.dma_start(dbg["dbg_wp"][:], wp_sb[:])
                if f == FT - 1:
                    pending.extend((outT, b, tt, ns) for tt in range(NQT) for ns in range(2))

            # ---- prologue: proj (b0, f0), bias replicate -----------------
            def emit_proj_full(xT, f):
                qt_ = qkpool.tile([128, N], F32R, tag="q", name="qTf")
                kt_ = qkpool.tile([128, N], F32R, tag="k", name="kTf")
                k16 = k16pool.tile([128, N], BF16, tag="k16", name="k16")
                vf = vfpool.tile([128, NKT * 128], BF16, tag="vf", name="vf")
                return qt_, kt_, k16, vf

            cur = emit_proj_full(xT0, 0)
            for c in range(2):
                emit_proj_chunk(xT0, 0, 0, c, cur[0], cur[1])
                emit_proj_chunk(xT0, 0, 1, c, cur[0], cur[1])
                emit_v_half(cur[1], cur[2], cur[3], c)
            emit_vf_build(cur[2], cur[3])

            for ns in range(2):
                pb = ps_proj.tile([128, 512], F32, tag="pp", name="pb")
                nc.tensor.matmul(
                    pb[:, 0:384],
                    bc_sb[0:1, 768:896],
                    bc_sb[0:1, 384 * ns : 384 * (ns + 1)],
                    start=True,
                    stop=True,
                )
                nc.vector.tensor_copy(bias_rep[:, 384 * ns : 384 * (ns + 1)], pb[:, 0:384])

            # ---- main loop ----------------------------------------------
            xT = xT0
            xT_next = None
            for b in range(BL):
                if b > 0:
                    xT = xT_next
                outT = outTpool.tile([128, FT, NQT, 128], BF16, tag="outT", name="outT")
                for f in range(FT):
                    qt_, kt_, k16, vf = cur
                    av_tiles = [
                        ps_av.tile([128, 390], F32, tag="av", name="av") for _ in range(3)
                    ]
                    # zero each av bank with one whole-bank start=True matmul
                    # (0 * anything); marks the 2KB zero region pending-zero and
                    # anchors a real write-dep for the start=False accumulates.
                    for g in range(3):
                        nc.tensor.matmul(
                            av_tiles[g][:],
                            zc_sb[0:1, 0:128],
                            zc_sb[0:1, 0:390],
                            start=True,
                            stop=False,
                            skip_group_check=True,
                        )
                    fin_ctx = (b, f, outT)

                    # filler schedule for this stage (next-stage projections
                    # and pending out-projection pieces)
                    fillers = deque()
                    nf = f + 1
                    if nf < FT or b + 1 < BL:
                        nxt_b = b if nf < FT else b + 1
                        nxt_f = nf % FT
                        nxt_x = xT if nf < FT else xT_next
                        nxt = emit_proj_full(nxt_x, nxt_f)
                        for c in range(2):
                            fillers.append(
                                (lambda c=c, nx=nxt_x, nf_=nxt_f, t=nxt: emit_proj_chunk(
                                    nx, nf_, 0, c, t[0], t[1]))
                            )
                            fillers.append(
                                (lambda c=c, nx=nxt_x, nf_=nxt_f, t=nxt: (
                                    emit_proj_chunk(nx, nf_, 1, c, t[0], t[1]),
                                    emit_v_half(t[1], t[2], t[3], c)))
                            )
                        fillers.append(lambda t=nxt: emit_vf_build(t[2], t[3]))
                    else:
                        nxt = None

                    if b == 0 and f == 1:
                        emit_wp_load()
                    if b == 0 and f == 2:
                        xT_next = emit_xT(1, sliced=False)

                    exp_split = b == 0 and f == 0  # ramp ACT before x c1 lands
                    if kdebug and b == 0 and f == 0:
                        nc.sync.dma_start(dbg["dbg_qT"][:], qt_[:])
                        nc.sync.dma_start(dbg["dbg_kT"][:], kt_[:])
                        nc.sync.dma_start(dbg["dbg_k16"][:], k16[:])
                        nc.sync.dma_start(dbg["dbg_vf"][:], vf[:])
                    for kt in range(NKT):
                        if b == 0 and f == 0 and kt == 4:
                            emit_weight_rest()
                        pT0 = pTpool.tile([128, N], BF16, tag="pT", name="pT0")
                        pT1 = pTpool.tile([128, N], BF16, tag="pT", name="pT1")
                        pTs = (pT0, pT1)
                        for e in range(2):
                            sct = ps_sc.tile([128, 1024], F32, tag="sc", name="sct")
                            for c in range(2):
                                nc.tensor.matmul(
                                    sct[:, 512 * c : 512 * (c + 1)],
                                    kt_[64 * e : 64 * (e + 1), 128 * kt : 128 * (kt + 1)],
                                    qt_[64 * e : 64 * (e + 1), 512 * c : 512 * (c + 1)],
                                    start=True,
                                    stop=True,
                                )
                                if exp_split:
                                    nc.scalar.activation(
                                        pTs[e][:, 512 * c : 512 * (c + 1)],
                                        sct[:, 512 * c : 512 * (c + 1)],
                                        EXP,
                                        scale=SCALE,
                                    )
                            if not exp_split:
                                nc.scalar.activation(pTs[e][:], sct[:], EXP, scale=SCALE)
                        if kdebug and b == 0 and f == 0 and kt == 0:
                            nc.sync.dma_start(dbg["dbg_pT0"][:], pT0[:])
                            nc.sync.dma_start(dbg["dbg_pT1"][:], pT1[:])
                        # mid-step filler keeps PE fed while ACT chews exps.
                        # Keep 5 pieces in reserve: they cover the tail's
                        # PE idle while the last stage normalizes.
                        if fillers:
                            fillers.popleft()()
                        elif len(pending) > (4 if (b == BL - 1 and f == FT - 1) else 12):
                            emit_outproj_piece(*pending.popleft())
                        if len(pend_av) > 2:
                            pend_av.popleft()()
                        pend_av.append(
                            lambda kt=kt, pT0=pT0, pT1=pT1, vf=vf, av=av_tiles, fc=fin_ctx: emit_av_group(
                                kt, pT0, pT1, vf, av, fc
                            )
                        )
                    cur = nxt

            # ---- tail ----------------------------------------------------
            while pend_av:
                pend_av.popleft()()
            flip = 0
            KINDS = (0, 1, 1, 2, 2)  # pp, sc, sc, av, av — 5-bank rotation
            while pending:
                emit_outproj_piece(*pending.popleft(), kind=KINDS[flip])
                flip = (flip + 1) % 5

    nc.compile()
    return nc


def _get_nc():
    if not _NC_CACHE:
        _NC_CACHE.append(_build())
    return _NC_CACHE[0]


def _to_np(a):
    try:
        return np.asarray(a)
    except Exception:
        import jax

        return np.asarray(jax.device_get(a))


def _prep_inputs(x, Wq, bq, Wk, bk, Wp, bp):
    import ml_dtypes

    x, Wq, bq, Wk, bk, Wp, bp = (_to_np(a) for a in (x, Wq, bq, Wk, bk, Wp, bp))
    x = np.ascontiguousarray(np.asarray(x, dtype=np.float32))
    wqt = np.ascontiguousarray(np.asarray(Wq, np.float32).T.astype(ml_dtypes.bfloat16))
    wkt = np.ascontiguousarray(np.asarray(Wk, np.float32).T.astype(ml_dtypes.bfloat16))
    wpt = np.ascontiguousarray(np.asarray(Wp, np.float32).T)
    bqp = np.ascontiguousarray(np.asarray(bq, np.float32).reshape(FT, 128).T)
    bkp = np.ascontiguousarray(np.asarray(bk, np.float32).reshape(FT, 128).T)
    bc = np.zeros((1, 896), np.float32)
    bc[0, :D] = np.asarray(bp, np.float32)
    bc[0, D : D + 128] = 1.0
    import ml_dtypes

    identb = np.eye(128, dtype=ml_dtypes.bfloat16)
    base = {
        "wqt": wqt, "wkt": wkt, "wpt": wpt,
        "bqp": bqp, "bkp": bkp, "bc": bc, "identb": identb,
    }
    in_maps = []
    for c in range(N_CORES):
        xc = np.ascontiguousarray(
            x[BL * c : BL * (c + 1)].reshape(T, D).T.astype(ml_dtypes.bfloat16)
        )
        in_maps.append(dict(base, x=xc))
    return in_maps


def _run(in_maps, **kw):
    from concourse.bass_utils import run_bass_kernel_spmd

    nc = _get_nc()
    return run_bass_kernel_spmd(nc, in_maps, core_ids=list(range(N_CORES)), **kw)


def kernel(x, Wq, bq, Wk, bk, Wp, bp):
    in_maps = _prep_inputs(x, Wq, bq, Wk, bk, Wp, bp)
    res = _run(in_maps)
    out = np.concatenate(
        [r["out"].reshape(BL, N, D) for r in res.results], axis=0
    )
    return out.astype(np.float32)


# revision 83
# speedup vs baseline: 1.0075x; 1.0010x over previous
"""Trainium2 Bass kernel for nn_AttentionSeparateQKV (B=16, N=1024, D=768, H=12).

Data-parallel over batch: 8 NeuronCores x 2 batches each. Per core, per
batch, per head-pair f (128 features = heads 2f, 2f+1):
  qT/kT projections (fp32r matmuls, bias fused in PSUM->SBUF add)
  scores computed TRANSPOSED (key-major): scT[key,q] = kT_h^T @ qT_h
  exp on ScalarE (scale fused) -> probsT bf16 directly (no probs transpose,
  no accum_out)
  V = K: vf[key,d] via 8 PE transposes per stage into a shared PSUM bank
  (the XBAR bf16 transpose corrupts even partitions on this backend for
  this src pattern); AV is emitted q-major (M=128 full) with an extra
  ap-1 matmul against a ones vector per region for softmax denominators:
    av[q, 0:64] = sum_k p*v,  av[q, 64] = sum_k p
  Multi-region PSUM accumulation uses one whole-bank start=True zeroing
  matmul per bank per stage (hardware pending-zero covers the 2KB zero
  region) + start=False region accumulates with skip_group_check.
  normalize on DVE with per-partition scalars (denom is per-q now)
  attn -> feature-major outT via one XBAR DMA transpose per stage
  out-proj bf16 matmuls + replicated-bias DVE add
"""

import sys

if "/opt/trn_rl_repo" not in sys.path:
    sys.path.insert(0, "/opt/trn_rl_repo")

from collections import deque

import numpy as np

B, N, D, H = 16, 1024, 768, 12
HD = D // H                # 64
SCALE = float(HD) ** -0.5  # 0.125
N_CORES = 8
BL = B // N_CORES          # 2 batches per core
T = BL * N                 # 2048 tokens per core
FT = D // 128              # 6 feature tiles == head pairs
NKT = N // 128             # 8 key tiles per batch
NQT = N // 128             # 8 query tiles per batch

_NC_CACHE = []


def _build():
    import concourse.mybir as mybir
    import concourse.tile as tile
    from concourse import bacc

    F32 = mybir.dt.float32
    F32R = mybir.dt.float32r
    BF16 = mybir.dt.bfloat16
    EXP = mybir.ActivationFunctionType.Exp
    ADD = mybir.AluOpType.add

    # Always-on: these tiny stage-(b0,f0) DRAM dumps add readers whose
    # dependencies steer the tile scheduler into an instruction order that
    # is correct on hardware (without them a latent ordering hazard in the
    # shared-PSUM-bank accumulation produces wrong results on this backend).
    kdebug = True

    nc = bacc.Bacc("TRN2", target_bir_lowering=False, debug=False)

    x_d = nc.dram_tensor("x", [D, T], BF16, kind="ExternalInput").ap()  # host-pretransposed
    wq_d = nc.dram_tensor("wqt", [D, D], BF16, kind="ExternalInput").ap()
    wk_d = nc.dram_tensor("wkt", [D, D], BF16, kind="ExternalInput").ap()
    wp_d = nc.dram_tensor("wpt", [D, D], F32R, kind="ExternalInput").ap()
    bq_d = nc.dram_tensor("bqp", [128, FT], F32, kind="ExternalInput").ap()
    bk_d = nc.dram_tensor("bkp", [128, FT], F32, kind="ExternalInput").ap()
    bc_d = nc.dram_tensor("bc", [1, 896], F32R, kind="ExternalInput").ap()
    id_d = nc.dram_tensor("identb", [128, 128], BF16, kind="ExternalInput").ap()
    out_d = nc.dram_tensor("out", [T, D], F32, kind="ExternalOutput").ap()
    dbg = {}
    if kdebug:
        for nm, shape, dt in [
            ("dbg_qT", [128, N], F32R),
            ("dbg_kT", [128, N], F32R),
            ("dbg_k16", [128, N], BF16),
            ("dbg_vf", [128, NKT, 128], BF16),
            ("dbg_pT0", [128, N], BF16),
            ("dbg_pT1", [128, N], BF16),
            ("dbg_avsb", [128, 3, 390], F32),
            ("dbg_attn", [128, NQT, 128], BF16),
            ("dbg_outT", [128, FT, NQT, 128], BF16),
            ("dbg_brep", [128, D], F32),
            ("dbg_wp", [128, FT, D], BF16),
        ]:
            dbg[nm] = nc.dram_tensor(nm, shape, dt, kind="ExternalOutput").ap()

    x_r = x_d.rearrange("(ko kp) t -> kp ko t", kp=128)

    with tile.TileContext(nc) as tc:
        with (
            tc.tile_pool(name="const", bufs=1) as cpool,
            tc.tile_pool(name="xp", bufs=2) as xpool,
            tc.tile_pool(name="qk", bufs=2) as qkpool,
            tc.tile_pool(name="k16p", bufs=2) as k16pool,
            tc.tile_pool(name="vfp", bufs=2) as vfpool,
            tc.tile_pool(name="pTp", bufs=9) as pTpool,
            tc.tile_pool(name="avsb", bufs=2) as avsbpool,
            tc.tile_pool(name="wpp", bufs=1) as wppool,
            tc.tile_pool(name="attnp", bufs=2) as attnpool,
            tc.tile_pool(name="outTp", bufs=2) as outTpool,
            tc.tile_pool(name="finp", bufs=3) as finpool,
            tc.tile_pool(name="recipp", bufs=6) as recippool,
            tc.tile_pool(name="ps_sc", bufs=2, space="PSUM") as ps_sc,
            tc.tile_pool(name="ps_av", bufs=3, space="PSUM") as ps_av,
            tc.tile_pool(name="ps_proj", bufs=1, space="PSUM") as ps_proj,
        ):
            # ---- constants / weights (gpsimd SWDGE: keeps the scalar/ACT
            # queue free of DMA dispatches, which block exp dispatch) ----
            bq_sb = cpool.tile([128, FT], F32, tag="bq")
            bk_sb = cpool.tile([128, FT], F32, tag="bk")
            bc_sb = cpool.tile([1, 896], F32R, tag="bc")
            zc_sb = cpool.tile([1, 1024], BF16, tag="zc")
            ones_sb = cpool.tile([128, 1], BF16, tag="ones")
            id_sb = cpool.tile([128, 128], BF16, tag="ident")

            wq_sb = cpool.tile([128, FT, D], BF16, tag="wq")
            wk_sb = cpool.tile([128, FT, D], BF16, tag="wk")
            wp_sb = cpool.tile([128, FT, D], BF16, tag="wp")
            bias_rep = cpool.tile([128, D], F32, tag="brep")
            wq_r = wq_d.rearrange("(ko kp) m -> kp ko m", kp=128)
            wk_r = wk_d.rearrange("(ko kp) m -> kp ko m", kp=128)

            def emit_xT(b, sliced):
                """Load feature-major x slice for batch b (host-pretransposed).
                sliced=True loads 12 (ko, half) pieces, first-needed first."""
                xT = xpool.tile([128, FT, N], BF16, tag="xT", name="xT")
                if sliced:
                    for ko in range(FT):
                        nc.sync.dma_start(
                            xT[:, ko : ko + 1, :],
                            x_r[:, ko : ko + 1, b * N : (b + 1) * N],
                        )
                else:
                    nc.gpsimd.dma_start(xT[:], x_r[:, :, b * N : (b + 1) * N])
                return xT

            # first needed pieces first, interleaved on the fast HWDGE queue:
            # wq f0, x ko0 (first proj matmul's inputs), wk f0, rest of x
            xT0 = xpool.tile([128, FT, N], BF16, tag="xT", name="xT0")
            nc.sync.dma_start(wq_sb[:, :, 0:128], wq_r[:, :, 0:128])
            nc.sync.dma_start(xT0[:, 0:1, :], x_r[:, 0:1, 0:N])
            nc.sync.dma_start(wk_sb[:, :, 0:128], wk_r[:, :, 0:128])
            for ko in range(1, FT):
                nc.sync.dma_start(
                    xT0[:, ko : ko + 1, :], x_r[:, ko : ko + 1, 0:N]
                )
            nc.gpsimd.dma_start(bq_sb[:], bq_d[:])
            nc.gpsimd.dma_start(bk_sb[:], bk_d[:])
            nc.gpsimd.dma_start(bc_sb[:], bc_d[:])
            nc.gpsimd.dma_start(wq_sb[:, :, 128:256], wq_r[:, :, 128:256])
            nc.gpsimd.dma_start(wk_sb[:, :, 128:256], wk_r[:, :, 128:256])
            nc.gpsimd.dma_start(id_sb[:], id_d[:])
            # exact constants built from loaded data (no memset dependence):
            # zc = bc*0 (zeros), ones = bq*0 + 1
            nc.vector.tensor_scalar_mul(zc_sb[:, 0:896], bc_sb[0:1, 0:896], 0.0)
            nc.vector.tensor_scalar_mul(zc_sb[:, 896:1024], bc_sb[0:1, 0:128], 0.0)
            nc.vector.tensor_scalar(
                ones_sb[:], bq_sb[:, 0:1], 0.0, 1.0,
                mybir.AluOpType.mult, mybir.AluOpType.add,
            )

            def emit_weight_rest():
                """Deferred f2..f5 weight slices — emitted mid-stage-f0 so the
                f0 v-transposes win the DMA queue race."""
                for wf in range(2, FT):
                    sl = slice(128 * wf, 128 * (wf + 1))
                    nc.gpsimd.dma_start(wq_sb[:, :, sl], wq_r[:, :, sl])
                    nc.gpsimd.dma_start(wk_sb[:, :, sl], wk_r[:, :, sl])

            wp_r = wp_d.rearrange("(ko kp) m -> kp ko m", kp=128)

            def emit_wp_load():
                """Deferred wp load + bf16 convert (needed only by the first
                out-projection, ~100us in). Converts run on the idle Pool
                engine so the DVE queue stays clear; the staging tile holds
                half of wp at a time."""
                for h in range(2):
                    wp32 = wppool.tile([128, 3, D], F32R, tag="wp32", name="wp32")
                    nc.gpsimd.dma_start(wp32[:], wp_r[:, 3 * h : 3 * (h + 1), :])
                    for ks in range(3):
                        nc.gpsimd.tensor_copy(
                            wp_sb[:, 3 * h + ks, :], wp32[:, ks, :]
                        )

            # ---- stage-state helpers -------------------------------------
            pend_av = deque()   # deferred AV-group closures
            pending = deque()   # deferred out-projection piece args (outT, b, tt, ns)
            fin_tiles = {}

            def emit_proj_chunk(xT, f, which, c, qt_, kt_):
                """One projection chunk: 6 matmuls + bias add for tokens
                [512c, 512c+512) of q (which=0) or k (which=1)."""
                w_sb = wq_sb if which == 0 else wk_sb
                b_sb = bq_sb if which == 0 else bk_sb
                dst = qt_ if which == 0 else kt_
                pp = ps_proj.tile([128, 512], F32, tag="pp", name="pp")
                for ks in range(FT):
                    nc.tensor.matmul(
                        pp[:],
                        w_sb[:, ks, 128 * f : 128 * (f + 1)],
                        xT[:, ks, 512 * c : 512 * (c + 1)],
                        start=(ks == 0),
                        stop=(ks == FT - 1),
                    )
                nc.vector.tensor_scalar_add(
                    dst[:, 512 * c : 512 * (c + 1)], pp[:], b_sb[:, f : f + 1]
                )

            def emit_v_half(kt_, k16, vf, c):
                """bf16 copy of kT half c (transpose happens separately on PE;
                the XBAR bf16 transpose corrupts even partitions on this HW)."""
                nc.vector.tensor_copy(
                    k16[:, 512 * c : 512 * (c + 1)], kt_[:, 512 * c : 512 * (c + 1)]
                )

            def emit_vf_build(k16, vf):
                """vf[key, kt, d] = k16[d, kt*128+key] via 8 PE transposes into
                one shared PSUM bank (zero-anchored), then one copy to SBUF."""
                vfT = ps_proj.tile([128, 512], F32, tag="pp", name="vfT")
                nc.tensor.matmul(
                    vfT[:], zc_sb[0:1, 0:128], zc_sb[0:1, 0:512],
                    start=True, stop=False, skip_group_check=True,
                )
                vfT16 = vfT.bitcast(BF16)
                for kt in range(NKT):
                    nc.tensor.matmul(
                        vfT16[:, 128 * kt : 128 * (kt + 1)],
                        k16[:, 128 * kt : 128 * (kt + 1)],
                        id_sb[:],
                        is_transpose=True,
                        start=False, stop=False, skip_group_check=True,
                    )
                nc.vector.tensor_copy(vf[:], vfT16[:])

            # (vf tiles are flat [128, 1024]: cols = kt*128 + d)

            def emit_outproj_piece(outT, b, tt, ns, kind=0):
                """Half (384 cols) of the final projection for token tile tt.
                kind=0 uses the shared proj bank; kind=1 borrows a scores
                tile (only safe in the tail when scoring is finished)."""
                if ns == 0:
                    fin_tiles[(b, tt)] = finpool.tile([128, D], F32, tag="fin", name="fin")
                fin = fin_tiles[(b, tt)]
                if kind == 0:
                    pf = ps_proj.tile([128, 512], F32, tag="pp", name="pf")
                elif kind == 1:
                    pf = ps_sc.tile([128, 1024], F32, tag="sc", name="pf")
                else:
                    # tail only: av banks are idle once the last avsb copies
                    # finish; their skip_group_check accumulates never set
                    # started_view, so a fresh start=True group is legal.
                    pf = ps_av.tile([128, 390], F32, tag="av", name="pf")
                for ks in range(FT):
                    nc.tensor.matmul(
                        pf[:, 0:384],
                        outT[:, ks, tt, :],
                        wp_sb[:, ks, 384 * ns : 384 * (ns + 1)],
                        start=(ks == 0),
                        stop=(ks == FT - 1),
                    )
                nc.vector.tensor_tensor(
                    fin[:, 384 * ns : 384 * (ns + 1)],
                    pf[:, 0:384],
                    bias_rep[:, 384 * ns : 384 * (ns + 1)],
                    ADD,
                )
                if ns == 1:
                    nc.sync.dma_start(
                        out_d[b * N + 128 * tt : b * N + 128 * (tt + 1), :], fin[:]
                    )

            QT_GROUPS = ((0, 1, 2), (3, 4, 5), (6, 7))

            def emit_av_group(kt, pT0, pT1, vf, av_tiles, fin_ctx):
                """AV matmuls for key tile kt (all query tiles, both heads).
                The bank was zeroed by a start=True whole-bank matmul at stage
                start (hardware pending-zero covers the full 2KB region), so
                every region accumulates with start=False; the group check is
                skipped since regions share the bank's zero region.
                On the last key tile, evacuate PSUM to SBUF (releases the
                av banks fast) and emit normalization + transpose."""
                pTs = (pT0, pT1)
                for qt in range(NQT):
                    g, r = qt // 3, qt % 3
                    for e in range(2):
                        base = 130 * r + 65 * e
                        nc.tensor.matmul(
                            av_tiles[g][:, base : base + 64],
                            pTs[e][:, 128 * qt : 128 * (qt + 1)],
                            vf[:, 128 * kt + 64 * e : 128 * kt + 64 * e + 64],
                            start=False,
                            stop=False,
                            skip_group_check=True,
                        )
                        nc.tensor.matmul(
                            av_tiles[g][:, base + 64 : base + 65],
                            pTs[e][:, 128 * qt : 128 * (qt + 1)],
                            ones_sb[:, 0:1],
                            start=False,
                            stop=False,
                            skip_group_check=True,
                        )
                if kt == NKT - 1:
                    av_sb = avsbpool.tile([128, 3, 390], F32, tag="avsb", name="av_sb")
                    nc.vector.tensor_copy(av_sb[:, 0, :], av_tiles[0][:])
                    nc.vector.tensor_copy(av_sb[:, 1, :], av_tiles[1][:])
                    nc.vector.tensor_copy(av_sb[:, 2, 0:260], av_tiles[2][:, 0:260])
                    if kdebug and fin_ctx[0] == 0 and fin_ctx[1] == 0:
                        nc.sync.dma_start(dbg["dbg_avsb"][:, 0:2, :], av_sb[:, 0:2, :])
                        nc.sync.dma_start(
                            dbg["dbg_avsb"][:, 2, 0:260], av_sb[:, 2, 0:260]
                        )
                    emit_normalize(av_sb, fin_ctx)

            def emit_normalize(av_sb, fin_ctx):
                """Per-stage softmax normalization + attn transpose to outT.
                On the very last stage, transpose per query tile so the tail
                out-projection can start as soon as its tokens are ready."""
                b, f, outT = fin_ctx
                last = False  # per-qt [128,128] transposes misbehave on HW
                attn = attnpool.tile([128, NQT, 128], BF16, tag="attn", name="attn")
                recips = []
                for g, qts in enumerate(QT_GROUPS):
                    cnt = 2 * len(qts)
                    rc = recippool.tile([128, 6], F32, tag="rc", name="rc")
                    nc.vector.reciprocal(
                        rc[:, 0:cnt], av_sb[:, g, 64 : 64 + 65 * (cnt - 1) + 1 : 65]
                    )
                    recips.append(rc)
                for qt in range(NQT):
                    g, r = qt // 3, qt % 3
                    for e in range(2):
                        nc.vector.tensor_scalar_mul(
                            attn[:, qt, 64 * e : 64 * (e + 1)],
                            av_sb[:, g, 130 * r + 65 * e : 130 * r + 65 * e + 64],
                            recips[g][:, 2 * r + e : 2 * r + e + 1],
                        )
                    if last:
                        nc.sync.dma_start(
                            outT[:, f, qt : qt + 1, :],
                            attn[:, qt, :],
                            transpose=True,
                        )
                if not last:
                    nc.sync.dma_start(outT[:, f, :, :], attn[:], transpose=True)
                if kdebug and b == 0 and f == 0:
                    nc.sync.dma_start(dbg["dbg_attn"][:], attn[:])
                if kdebug and b == 0 and f == FT - 1:
                    nc.sync.dma_start(dbg["dbg_outT"][:], outT[:])
                    nc.sync.dma_start(dbg["dbg_brep"][:], bias_rep[:])
                    nc.sync.dma_start(dbg["dbg_wp"][:], wp_sb[:])
                if f == FT - 1:
                    pending.extend((outT, b, tt, ns) for tt in range(NQT) for ns in range(2))

            # ---- prologue: proj (b0, f0), bias replicate -----------------
            def emit_proj_full(xT, f):
                qt_ = qkpool.tile([128, N], F32R, tag="q", name="qTf")
                kt_ = qkpool.tile([128, N], F32R, tag="k", name="kTf")
                k16 = k16pool.tile([128, N], BF16, tag="k16", name="k16")
                vf = vfpool.tile([128, NKT * 128], BF16, tag="vf", name="vf")
                return qt_, kt_, k16, vf

            cur = emit_proj_full(xT0, 0)
            for c in range(2):
                emit_proj_chunk(xT0, 0, 0, c, cur[0], cur[1])
                emit_proj_chunk(xT0, 0, 1, c, cur[0], cur[1])
                emit_v_half(cur[1], cur[2], cur[3], c)
            emit_vf_build(cur[2], cur[3])

            for ns in range(2):
                pb = ps_proj.tile([128, 512], F32, tag="pp", name="pb")
                nc.tensor.matmul(
                    pb[:, 0:384],
                    bc_sb[0:1, 768:896],
                    bc_sb[0:1, 384 * ns : 384 * (ns + 1)],
                    start=True,
                    stop=True,
                )
                nc.vector.tensor_copy(bias_rep[:, 384 * ns : 384 * (ns + 1)], pb[:, 0:384])

            # ---- main loop ----------------------------------------------
            xT = xT0
            xT_next = None
            for b in range(BL):
                if b > 0:
                    xT = xT_next
                outT = outTpool.tile([128, FT, NQT, 128], BF16, tag="outT", name="outT")
                for f in range(FT):
                    qt_, kt_, k16, vf = cur
                    av_tiles = [
                        ps_av.tile([128, 390], F32, tag="av", name="av") for _ in range(3)
                    ]
                    # zero each av bank with one whole-bank start=True matmul
                    # (0 * anything); marks the 2KB zero region pending-zero and
                    # anchors a real write-dep for the start=False accumulates.
                    for g in range(3):
                        nc.tensor.matmul(
                            av_tiles[g][:],
                            zc_sb[0:1, 0:128],
                            zc_sb[0:1, 0:390],
                            start=True,
                            stop=False,
                            skip_group_check=True,
                        )
                    fin_ctx = (b, f, outT)

                    # filler schedule for this stage (next-stage projections
                    # and pending out-projection pieces)
                    fillers = deque()
                    nf = f + 1
                    if nf < FT or b + 1 < BL:
                        nxt_b = b if nf < FT else b + 1
                        nxt_f = nf % FT
                        nxt_x = xT if nf < FT else xT_next
                        nxt = emit_proj_full(nxt_x, nxt_f)
                        for c in range(2):
                            fillers.append(
                                (lambda c=c, nx=nxt_x, nf_=nxt_f, t=nxt: emit_proj_chunk(
                                    nx, nf_, 0, c, t[0], t[1]))
                            )
                            fillers.append(
                                (lambda c=c, nx=nxt_x, nf_=nxt_f, t=nxt: (
                                    emit_proj_chunk(nx, nf_, 1, c, t[0], t[1]),
                                    emit_v_half(t[1], t[2], t[3], c)))
                            )
                        fillers.append(lambda t=nxt: emit_vf_build(t[2], t[3]))
                    else:
                        nxt = None

                    if b == 0 and f == 1:
                        emit_wp_load()
                    if b == 0 and f == 2:
                        xT_next = emit_xT(1, sliced=False)

                    exp_split = b == 0 and f == 0  # ramp ACT before x c1 lands
                    if kdebug and b == 0 and f == 0:
                        nc.sync.dma_start(dbg["dbg_qT"][:], qt_[:])
                        nc.sync.dma_start(dbg["dbg_kT"][:], kt_[:])
                        nc.sync.dma_start(dbg["dbg_k16"][:], k16[:])
                        nc.sync.dma_start(dbg["dbg_vf"][:], vf[:])
                    for kt in range(NKT):
                        if b == 0 and f == 0 and kt == 4:
                            emit_weight_rest()
                        pT0 = pTpool.tile([128, N], BF16, tag="pT", name="pT0")
                        pT1 = pTpool.tile([128, N], BF16, tag="pT", name="pT1")
                        pTs = (pT0, pT1)
                        for e in range(2):
                            sct = ps_sc.tile([128, 1024], F32, tag="sc", name="sct")
                            for c in range(2):
                                nc.tensor.matmul(
                                    sct[:, 512 * c : 512 * (c + 1)],
                                    kt_[64 * e : 64 * (e + 1), 128 * kt : 128 * (kt + 1)],
                                    qt_[64 * e : 64 * (e + 1), 512 * c : 512 * (c + 1)],
                                    start=True,
                                    stop=True,
                                )
                                if exp_split:
                                    nc.scalar.activation(
                                        pTs[e][:, 512 * c : 512 * (c + 1)],
                                        sct[:, 512 * c : 512 * (c + 1)],
                                        EXP,
                                        scale=SCALE,
                                    )
                            if not exp_split:
                                nc.scalar.activation(pTs[e][:], sct[:], EXP, scale=SCALE)
                        if kdebug and b == 0 and f == 0 and kt == 0:
                            nc.sync.dma_start(dbg["dbg_pT0"][:], pT0[:])
                            nc.sync.dma_start(dbg["dbg_pT1"][:], pT1[:])
                        # mid-step filler keeps PE fed while ACT chews exps.
                        # Keep 5 pieces in reserve: they cover the tail's
                        # PE idle while the last stage normalizes.
                        if fillers:
                            fillers.popleft()()
                        elif len(pending) > (4 if (b == BL - 1 and f == FT - 1) else 14):
                            emit_outproj_piece(*pending.popleft())
                        if len(pend_av) > 2:
                            pend_av.popleft()()
                        pend_av.append(
                            lambda kt=kt, pT0=pT0, pT1=pT1, vf=vf, av=av_tiles, fc=fin_ctx: emit_av_group(
                                kt, pT0, pT1, vf, av, fc
                            )
                        )
                    cur = nxt

            # ---- tail ----------------------------------------------------
            while pend_av:
                pend_av.popleft()()
            flip = 0
            KINDS = (0, 1, 1, 2, 2)  # pp, sc, sc, av, av — 5-bank rotation
            while pending:
                emit_outproj_piece(*pending.popleft(), kind=KINDS[flip])
                flip = (flip + 1) % 5

    nc.compile()
    return nc


def _get_nc():
    if not _NC_CACHE:
        _NC_CACHE.append(_build())
    return _NC_CACHE[0]


def _to_np(a):
    try:
        return np.asarray(a)
    except Exception:
        import jax

        return np.asarray(jax.device_get(a))


def _prep_inputs(x, Wq, bq, Wk, bk, Wp, bp):
    import ml_dtypes

    x, Wq, bq, Wk, bk, Wp, bp = (_to_np(a) for a in (x, Wq, bq, Wk, bk, Wp, bp))
    x = np.ascontiguousarray(np.asarray(x, dtype=np.float32))
    wqt = np.ascontiguousarray(np.asarray(Wq, np.float32).T.astype(ml_dtypes.bfloat16))
    wkt = np.ascontiguousarray(np.asarray(Wk, np.float32).T.astype(ml_dtypes.bfloat16))
    wpt = np.ascontiguousarray(np.asarray(Wp, np.float32).T)
    bqp = np.ascontiguousarray(np.asarray(bq, np.float32).reshape(FT, 128).T)
    bkp = np.ascontiguousarray(np.asarray(bk, np.float32).reshape(FT, 128).T)
    bc = np.zeros((1, 896), np.float32)
    bc[0, :D] = np.asarray(bp, np.float32)
    bc[0, D : D + 128] = 1.0
    import ml_dtypes

    identb = np.eye(128, dtype=ml_dtypes.bfloat16)
    base = {
        "wqt": wqt, "wkt": wkt, "wpt": wpt,
        "bqp": bqp, "bkp": bkp, "bc": bc, "identb": identb,
    }
    in_maps = []
    for c in range(N_CORES):
        xc = np.ascontiguousarray(
            x[BL * c : BL * (c + 1)].reshape(T, D).T.astype(ml_dtypes.bfloat16)
        )
        in_maps.append(dict(base, x=xc))
    return in_maps


def _run(in_maps, **kw):
    from concourse.bass_utils import run_bass_kernel_spmd

    nc = _get_nc()
    return run_bass_kernel_spmd(nc, in_maps, core_ids=list(range(N_CORES)), **kw)


def kernel(x, Wq, bq, Wk, bk, Wp, bp):
    in_maps = _prep_inputs(x, Wq, bq, Wk, bk, Wp, bp)
    res = _run(in_maps)
    out = np.concatenate(
        [r["out"].reshape(BL, N, D) for r in res.results], axis=0
    )
    return out.astype(np.float32)
